# revision 1
# baseline (speedup 1.0000x reference)
"""Trainium2 Bass kernel for nn_Net_4200478015619 (dense_mlp).

59.4us baseline -> 52.4us: dual-engine PSUM drain, TT 2x finish, w_sum
head for the alpha-term, header-rides-x-stream DMA, staggered transfers,
exit-tail semaphore hygiene elided (NEFF reloads per call).

Dead ends verified on hardware (don't retry without new information):
 * Custom-DVE 2x_1p uop for the cube: the uops_2x table path generates
   and loads fine (runs at 1x), but ANY instruction with perf_max!=0 on
   the CUSTOM_DVE_ANT dispatch wedges the NeuronCore (unrecoverable) --
   the NX firmware handler appears not to implement mode dispatch for
   custom rows. Stock TensorTensor 2x works (used for the D3 finish).
 * MM2 col-group tiling does not overlap trio members (per-matmul
   LDWEIGHTS serializes on the shared fg/bg weight buffer).
 * GpSimd TensorScalarPtr is rejected by walrus; GpSimd cannot read PSUM.
 * ZW=1536 with 2 PSUM z-buffers stalls on buffer rotation (58.8us);
   all-upfront concurrent x transfers collapse DRAM locality (~187GB/s).
 * Pre-TileContext DMA issue deadlocks the Tile scheduler's internal sim.

Computes, for x (262144, 128) fp32 and W (100, 128) fp32:
    z   = x @ W.T                        # (B, 100)
    y   = z**3 + 0.1 * z
    out = sum(y, axis=1, keepdims=True)  # (B, 1)

Sharding: pure data parallel over 8 NeuronCores; core c gets rows
[c*32768, (c+1)*32768), transposed on host to xT (128, 32768) bf16.

Architecture (per core), trace-driven:
 * MM1 streams x chunks against a stationary W_aug [128, 101] whose
   101st column is sum_m W[m] — so z[100] = sum_m z_m, carrying the
   whole alpha-term through the pipeline linearly.
 * The pointwise cubic drains PSUM through BOTH PSUM-capable engines:
     D1 tiles: DVE custom CUBE op (fused drain + (z^2+a)*z), 1x mode
     D3 tiles: ACT Copy + ACT Square drain to SBUF bf16, then a raw
               InstTensorTensor multiply zc *= sq on DVE (2x_1p bf16);
               the alpha-term rides row 100 (raw z_sum) weighted 0.1
               by that chunk's selector column in MM2.
 * MM2 scatters each chunk's head-sum into one [96, 512] PSUM bank via
   per-chunk one-hot selector columns (widths 22) at column groups
   32*(c%3); single eviction + DMA at the end.
 * MM2 emission lags the drains by MM2_LAG z-tiles so the PE FIFO never
   blocks on a pending pointwise op.
 * Input DMA: escalating transfer sizes on the sync HWDGE queue.
"""

import numpy as np

import concourse.bacc as bacc
import concourse.mybir as mybir
import concourse.tile as tile
from concourse.bass_utils import run_bass_kernel_spmd

# --- TileContext exit-drain legalization (same as baseline) ----------------
from concourse.vector_clock import ScopedClock, VectorClock


def _patched_drain_and_barrier(self, tick_clock, wait_clock):
    g = tick_clock.global_clock
    n = len(g)
    pending = [i for i in range(n) if g[i] > 0]
    engines = [e for e in self.nc.engines.values()]
    for k, p in enumerate(pending):
        vec = [0] * n
        vec[p] = g[p]
        eng = engines[k % len(engines)]
        nop_inst = eng.nop()
        wait_clock.add_sem_waits(nop_inst.ins, ScopedClock({None: VectorClock(vec)}))
    self.nc.sync.drain()
    self.nc.all_engine_barrier()
    assert self.sems is not None
    popped = self.nc._tile_sem_poison_stack.pop()
    assert popped is self._sem_poison
    # Exit semaphore hygiene (device-side clears + a second barrier) is only
    # needed if the loaded NEFF re-executes without a reload; this harness
    # rebuilds and reloads per kernel() call (entry already relies on
    # load-time-zeroed semaphores), so free the handles host-side only and
    # skip ~1-2us of exit-tail device work.
    self.nc._state.prepend_free_semaphores(
        [s.num for s in self.sems.allocated().values()]
    )


tile.TileContext._drain_and_barrier = _patched_drain_and_barrier
# ---------------------------------------------------------------------------


N_CORES = 8
B = 262144
B_CORE = B // N_CORES  # 32768
F = 128
M = 100
MA = M + 1                      # heads + z_sum row
ALPHA = 0.1
CHUNK = 512
ZW = 1024                       # z-tile width: 2 chunks, 2 PSUM banks
CPT = ZW // CHUNK
N_ZT = B_CORE // ZW             # 32
N_CHUNKS = B_CORE // CHUNK      # 64
SLOTS = 22                      # output slots per column group (ceil(64/3))
MM2_LAG = 7                     # z-tiles of lag before MM2 emission
N_WARMUP = 12
XWIDTHS = [2048, 2048, 4096, 4096, 4096, 8192, 8192]
assert sum(XWIDTHS) == B_CORE
# D3 tiles (ACT Copy+Square drain, DVE 2x TT finish): ~13 of 32, spread,
# none among the final tiles (keeps the tail on the short DVE path)
D3_TILES = frozenset({1, 3, 5, 8, 10, 12, 15, 17, 19, 22, 24, 26, 28})

_CUBE_OP = None


def _register_cube_op():
    """out = (Src0^2 + c0) * Src0  as one DVE instruction (1x mode)."""
    global _CUBE_OP
    if _CUBE_OP is not None:
        return _CUBE_OP
    import concourse.dve_ops as dve_ops
    from concourse.dve_spec import Spec, Src0, C0, sq, lower
    from concourse.dve_uop import DveOpSpec

    name = "CUBE_AXPB_ANT"
    for op in dve_ops.OPS:
        if op.name == name:
            _CUBE_OP = op
            return op
    spec = Spec(
        body=(sq(Src0) + C0) * Src0,
        reference=lambda in0, in1, s0, s1, imm2: (
            (in0.astype(np.float32) ** 2 + s0) * in0.astype(np.float32)
        ).astype(np.float32),
    )
    row = dve_ops._CUSTOM_DVE_ROW_BASE + len(dve_ops.OPS)
    assert row < 0x20, "custom-DVE opcode rows exhausted"
    shas = {
        ver: DveOpSpec(
            name=name, opcode=row, uops=lower(spec, ver=ver), rd1_en=False
        ).sha(ver)
        for ver in ("v3", "v4")
    }
    op = dve_ops.DveOp(name, spec, subdim=False, uops_sha=shas)
    dve_ops.OPS.append(op)
    dve_ops._SUB_OPCODE_FOR_NAME[name] = row
    dve_ops.CUSTOM_DVE_SPECS[name] = spec
    _CUBE_OP = op
    return op


def _tensor_tensor(eng, out, in0, in1, op):
    """Raw InstTensorTensor (bass exposes no helper): out = in0 <op> in1.
    The TT ISA op has a 2x_1p uop in the stock engine tables (bf16)."""
    return eng.add_instruction(
        mybir.InstTensorTensor(
            name=eng.bass.get_next_instruction_name(),
            op=op,
            ins=[eng.lower_ap(in0), eng.lower_ap(in1)],
            outs=[eng.lower_ap(out)],
        )
    )


def build_nc():
    cube_op = _register_cube_op()
    nc = bacc.Bacc()
    # xt carries [W_aug | selector table | x] so the weights+selectors ride
    # the head of the single input stream (a separate small-line DMA gets
    # starved behind the x flood for ~30us and head-of-line-blocks the PE).
    # Selector layout: col 22c + (c//3) is "hot": rows 0..99 = 1.0, row 100 =
    # 0.1 for D3 chunks (alpha * z_sum), 0 for D1.
    HDR = MA + N_CHUNKS * SLOTS  # 101 + 1408
    xt = nc.declare_dram_parameter(
        "xt", [F, HDR + B_CORE], mybir.dt.bfloat16, isOutput=False
    )
    out = nc.declare_dram_parameter("out", [96, CHUNK], mybir.dt.float32, isOutput=True)

    with tile.TileContext(nc) as tc:
        with (
            tc.tile_pool(name="wpool", bufs=1) as wpool,
            tc.tile_pool(name="xpool", bufs=5) as xpool,
            tc.tile_pool(name="ypool", bufs=2 * MM2_LAG + 4) as ypool,
            tc.tile_pool(name="sqpool", bufs=3) as sqpool,
            tc.tile_pool(name="opool", bufs=1) as opool,
            tc.tile_pool(name="zpsum", bufs=3, space="PSUM") as zpsum,
            tc.tile_pool(name="opsum", bufs=1, space="PSUM") as opsum,
        ):
            # header transfer: W_aug + selector table, first on the ring
            hdr = wpool.tile([F, HDR], mybir.dt.bfloat16)
            nc.sync.dma_start(out=hdr[:], in_=xt[:, 0:HDR])
            ws = hdr[:, 0:MA]

            def sel_slice(c):
                a = MA + SLOTS * c
                return hdr[0:MA, a : a + SLOTS]

            # HAM warm-up
            wu_w = wpool.tile([F, F], mybir.dt.bfloat16)
            nc.vector.memset(wu_w[:], 0.0)
            wu_x = wpool.tile([F, CHUNK], mybir.dt.bfloat16)
            nc.vector.memset(wu_x[:], 0.0)
            wu_p = zpsum.tile([MA, ZW], mybir.dt.float32, tag="zt")
            for _ in range(N_WARMUP):
                nc.tensor.matmul(
                    wu_p[:, :CHUNK], lhsT=wu_w[:, :MA], rhs=wu_x[:], start=True,
                    stop=True,
                )

            # x input stream on the sync HWDGE queue
            xtiles = []  # (start_col, width, tile)
            col = 0
            for w in XWIDTHS:
                xs = xpool.tile([F, w], mybir.dt.bfloat16, tag="xs")
                nc.sync.dma_start(out=xs[:], in_=xt[:, HDR + col : HDR + col + w])
                xtiles.append((col, w, xs))
                col += w

            def x_slice(c):
                a = c * CHUNK
                for start, w, xs in xtiles:
                    if start <= a and a + CHUNK <= start + w:
                        return xs[:, a - start : a - start + CHUNK]
                raise AssertionError(c)

            o_acc = opsum.tile([96, CHUNK], mybir.dt.float32)
            # unused scatter rows are never matmul-written; zero them so the
            # final whole-tile eviction reads initialized PSUM
            nc.vector.memset(o_acc[:], 0.0)
            y_of_chunk = {}  # chunk idx -> (tile holding y, col offset)
            next_mm2 = 0

            def emit_mm2(upto):
                nonlocal next_mm2
                while next_mm2 < upto and (
                    next_mm2 + 3 <= upto or upto == N_CHUNKS
                ):
                    hi = min(next_mm2 + 3, upto)
                    for c in range(next_mm2, hi):
                        g, s = c % 3, c // 3
                        yt, off = y_of_chunk.pop(c)
                        nc.tensor.matmul(
                            o_acc[32 * g : 32 * g + SLOTS, :],
                            lhsT=sel_slice(c),
                            rhs=yt[:, off : off + CHUNK],
                            start=(c == 0),
                            stop=(c == N_CHUNKS - 1),
                            tile_position=(0, 32 * g),
                            skip_group_check=True,
                        )
                    next_mm2 = hi

            for t in range(N_ZT):
                zt = zpsum.tile([MA, ZW], mybir.dt.float32, tag="zt")
                for k in range(CPT):
                    nc.tensor.matmul(
                        zt[:, k * CHUNK : (k + 1) * CHUNK],
                        lhsT=ws,
                        rhs=x_slice(t * CPT + k),
                        start=True,
                        stop=True,
                    )
                y = ypool.tile([MA, ZW], mybir.dt.bfloat16, tag="y")
                if t in D3_TILES:
                    # ACT drains z (raw, incl. z_sum row) and z^2; DVE then
                    # multiplies in place at 2x: y[0:100] = z * z^2
                    nc.scalar.copy(y[:], zt[:])
                    sq_t = sqpool.tile([M, ZW], mybir.dt.bfloat16, tag="sq")
                    nc.scalar.square(sq_t[:], zt[0:M, :])
                    _tensor_tensor(
                        nc.vector, y[0:M, :], y[0:M, :], sq_t[:],
                        mybir.AluOpType.mult,
                    )
                else:
                    # fused drain+cubic on DVE (1x from PSUM); row 100 gets
                    # (z_sum^2+a)*z_sum which the selector kills (row100=0)
                    nc.vector._custom_dve(cube_op, out=y[:], in0=zt[:], s0=ALPHA)
                for k in range(CPT):
                    y_of_chunk[t * CPT + k] = (y, k * CHUNK)
                if t >= MM2_LAG:
                    emit_mm2((t - MM2_LAG + 1) * CPT)
            emit_mm2(N_CHUNKS)

            osb = opool.tile([96, CHUNK], mybir.dt.float32)
            nc.scalar.copy(osb[:], o_acc[:])
            nc.sync.dma_start(out=out[:], in_=osb[:])
    nc.finalize()
    return nc


def _host_inputs(x, W):
    import ml_dtypes

    x = np.ascontiguousarray(x, dtype=np.float32)
    W = np.ascontiguousarray(W, dtype=np.float32)
    wa = np.concatenate([W.T, W.sum(axis=0, keepdims=True).T], axis=1)  # (128, 101)
    wt_np = np.ascontiguousarray(wa.astype(ml_dtypes.bfloat16))

    sel_np = np.zeros((F, N_CHUNKS * SLOTS), dtype=ml_dtypes.bfloat16)
    for c in range(N_CHUNKS):
        col = SLOTS * c + c // 3
        sel_np[0:M, col] = 1.0
        if (c // CPT) in D3_TILES:
            sel_np[M, col] = ALPHA
    wt_pad = np.zeros((F, MA), dtype=ml_dtypes.bfloat16)
    wt_pad[:, :] = wt_np

    in_maps = []
    for ci in range(N_CORES):
        shard = x[ci * B_CORE : (ci + 1) * B_CORE, :]
        xt_np = np.ascontiguousarray(shard.T.astype(ml_dtypes.bfloat16))
        in_maps.append(
            {"xt": np.ascontiguousarray(np.concatenate([wt_pad, sel_np, xt_np], axis=1))}
        )
    return in_maps


def _unscatter(out96):
    full = np.empty((N_CHUNKS, CHUNK), dtype=np.float32)
    for c in range(N_CHUNKS):
        full[c] = out96[32 * (c % 3) + c // 3]
    return full.reshape(B_CORE, 1)


def _run(x, W, trace=False, **run_kwargs):
    in_maps = _host_inputs(x, W)
    nc = build_nc()
    res = run_bass_kernel_spmd(
        nc, in_maps, list(range(N_CORES)), trace=trace, **run_kwargs
    )
    outs = [_unscatter(res.results[c]["out"]) for c in range(N_CORES)]
    full = np.concatenate(outs, axis=0)
    return full, res


def kernel(x, W):
    full, _ = _run(x, W)
    return full

